# revision 36
# baseline (speedup 1.0000x reference)
"""Transformer-XL relative multi-head attention, 8-way sharded on Trainium2.

Self-contained harness entry: kernel(**inputs) -> np.ndarray [4, 1024, 1024].

Sharding: core c handles batch b = c//2 and head-half hh = c%2 (8 of 16
heads). Each core computes a partial output (its heads' contribution
through Wo); the host unshard sums the two partials per batch (row-parallel
tensor parallelism for the output projection).

Attention flow (per head, per 128-row query tile):
  - position term M = (qh+v_bias) @ rh2^T  -> HBM shear (write stride W,
    read stride W+1) gives the circulant-shifted bd in [q, keys] layout.
  - per 128-key tile: PE-transpose of the bd tile into PSUM (start=True),
    then the content term k^T q accumulated on top (start=False), so the
    PSUM holds score^T = (ac+bd)^T without any DVE adds or DMA transposes.
  - exp on the Scalar engine straight out of PSUM into att^T in SBUF.
  - AV contracts key tiles; vh carries an extra ones-row so row 64 of the
    AV output is the softmax denominator; a 1-partition PE matmul
    broadcasts the reciprocal row and the AV evacuation multiplies by it.
"""

import os
import sys

sys.path.insert(0, "/opt/trn_rl_repo")

import numpy as np


import concourse.bass as bass
import concourse.mybir as mybir
from concourse.tile import TileContext, ScopedClock

F32 = mybir.dt.float32
F16 = mybir.dt.float16
AF = mybir.ActivationFunctionType
OP = mybir.AluOpType

S, T, D, HC, DK, P = 1024, 2048, 1024, 8, 64, 128
DH = HC * DK  # 512, head-slice width per core
NQT = S // P  # 8 query tiles
WMAX = 2048 + 128  # padded shear slot width (>= max W = 2175)
SLOT = P * WMAX + P  # dram scratch slot elements (pad for strided read)
EXP_BIAS = -7.0


def _patched_drain_and_barrier(self, tick_clock, wait_clock):
    # The walrus build in this container caps sync-waits per instruction;
    # Tile's stock tail drain carries one wait per live proc. Emit one SP nop
    # per wait instead, then the drain.
    dummy = mybir.InstNoOp(name="drain-wait-probe", ins=[], outs=[])
    dummy.engine = mybir.EngineType.SP
    wait_clock.add_sem_waits(dummy, ScopedClock({None: tick_clock.global_clock}))
    waits = []
    if dummy.sync_info is not None and dummy.sync_info.on_wait:
        waits = [(w.ant_name, w.wait_value) for w in dummy.sync_info.on_wait]
    assert self.sems is not None
    name2sem = {h.name: h for h in self.sems.allocated().values()}
    for name, val in waits:
        self.nc.sync.nop().wait_op(name2sem[name], val, "sem-ge")
    self.nc.sync.drain()
    self.nc.all_engine_barrier()
    popped = self.nc._tile_sem_poison_stack.pop()
    assert popped is self._sem_poison
    self.nc.clear_and_free_semaphores(list(self.sems.allocated().values()))
    self.nc.all_engine_barrier()


TileContext._drain_and_barrier = _patched_drain_and_barrier



def _split_multi_waits(nc, max_waits=1):
    """Walrus in this container rejects instructions carrying more than a
    couple of sync waits. Hoist extras onto same-engine NoOps just before
    the instruction (sequential on the engine, so semantics unchanged)."""
    for f in nc.m.functions:
        for bb in f.blocks:
            out = []
            changed = False
            for inst in bb.instructions:
                si = inst.sync_info
                if si is not None and si.on_wait and len(si.on_wait) > max_waits:
                    waits = list(si.on_wait)
                    for j, w in enumerate(waits[:-max_waits]):
                        nop = mybir.InstNoOp(
                            name=f"{inst.name}-wsplit{j}", ins=[], outs=[])
                        nop.engine = inst.engine
                        nop.sync_info = mybir.SyncInfo(on_wait=[w], on_update=[])
                        out.append(nop)
                    inst.sync_info = mybir.SyncInfo(
                        on_wait=waits[-max_waits:],
                        on_update=list(si.on_update))
                    changed = True
                out.append(inst)
            if changed:
                bb.instructions = out


def kq_of(qi):  # valid key count for query tile qi (keys j <= i + 1024)
    return (qi + 9) * P


def build_nc(split_waits=True):
    nc = bass.Bass(target_bir_lowering=True)

    qT = nc.declare_dram_parameter("qT", [D, S], F16, isOutput=False)
    kT = nc.declare_dram_parameter("kT", [D, T], F16, isOutput=False)
    vT = nc.declare_dram_parameter("vT", [D, T], F16, isOutput=False)
    RT = nc.declare_dram_parameter("RT", [D, T], F16, isOutput=False)
    Wq = nc.declare_dram_parameter("Wq", [D, DH], F16, isOutput=False)
    Wk = nc.declare_dram_parameter("Wk", [D, DH], F16, isOutput=False)
    Wv = nc.declare_dram_parameter("Wv", [D, DH], F16, isOutput=False)
    Wr = nc.declare_dram_parameter("Wr", [D, DH], F16, isOutput=False)
    Wo16 = nc.declare_dram_parameter("Wo16", [DH, D], F16, isOutput=False)
    ub = nc.declare_dram_parameter("ub", [P, 4], F32, isOutput=False)
    vb = nc.declare_dram_parameter("vb", [P, 4], F32, isOutput=False)
    ident = nc.declare_dram_parameter("ident", [P, P], F16, isOutput=False)
    masklo = nc.declare_dram_parameter(
        "masklo", [P, P], mybir.dt.uint8, isOutput=False)
    outp = nc.declare_dram_parameter("out", [S, D], F32, isOutput=True)

    with TileContext(nc) as tc:
        with (
            tc.tile_pool(name="persist", bufs=1) as pp,
            tc.tile_pool(name="consts", bufs=1) as cp,
        ):
            # persistent fp16 tensors (partition = dk within head-pair tile).
            # Matmuls only hit full rate with K=128 and M=128, so per-head
            # stationaries are zero-padded to all 128 partitions: the padded
            # rows multiply the other head's moving rows by zero.
            quTp0 = pp.tile([P, 4 * S], F16)    # (qh+u).T head-even, rows 64+ zero
            quTp1 = pp.tile([P, 4 * S], F16)    # head-odd, rows 0-63 zero
            qvTp0 = pp.tile([P, 4 * S], F16)    # (qh+v).T head-even, rows 64+ zero
            qvTp1 = pp.tile([P, 4 * S], F16)    # head-odd, rows 0-63 zero
            khT = pp.tile([P, 4 * T], F16)
            rh2T = pp.tile([P, 4 * 3072], F16)
            vh128 = pp.tile([P, 16 * (HC * P)], F16)  # tile block: 8 heads x 128
            concatT = pp.tile([P, 4 * S], F16)

            ub_sb = cp.tile([P, 4], F32)
            vb_sb = cp.tile([P, 4], F32)
            vbmu_sb = cp.tile([P, 4], F32)
            ident_sb = cp.tile([P, P], F16)
            masklo_sb = cp.tile([P, P], mybir.dt.uint8)
            zeros16 = cp.tile([P, P], F16)
            ones16 = cp.tile([P, DK], F16)
            expb_sb = cp.tile([P, 1], F32)
            nc.vector.memset(expb_sb[:], EXP_BIAS)
            nc.vector.memset(zeros16[:], 0.0)
            nc.vector.memset(ones16[:], 1.0)

            nc.sync.dma_start(out=ub_sb[:], in_=ub[:])
            nc.sync.dma_start(out=vb_sb[:], in_=vb[:])
            nc.vector.tensor_tensor(vbmu_sb[:], vb_sb[:], ub_sb[:], OP.subtract)
            nc.sync.dma_start(out=ident_sb[:], in_=ident[:])
            nc.sync.dma_start(out=masklo_sb[:], in_=masklo[:])
            nc.vector.memset(quTp0[DK:P, :], 0.0)
            nc.vector.memset(quTp1[0:DK, :], 0.0)
            nc.vector.memset(qvTp0[DK:P, :], 0.0)
            nc.vector.memset(qvTp1[0:DK, :], 0.0)
            nc.vector.memset(vh128[:], 0.0)

            # ---------------- projections ----------------
            def load_w(pool, wparam):
                wsb = pool.tile([P, 8 * DH], F16, tag="wsb")
                nc.sync.dma_start(
                    out=bass.AP(wsb.tensor, wsb.offset,
                                [[wsb.tensor.shape[1], P], [DH, 8], [1, DH]]),
                    in_=bass.AP(wparam, 0, [[DH, P], [P * DH, 8], [1, DH]]),
                )
                return wsb

            # qhT-style projection: out[512, ncols] = W_s @ xT, evacuated by fn
            def proj_T(pool, psum, wsb, xparam, ncols, evac):
                nth = ncols // 1024
                for th in range(nth):
                    psums = {k: psum.tile([P, 512], F32, tag="proj", name="projps")
                             for k in [(d, t2) for d in range(4) for t2 in range(2)]}
                    for kd in range(8):
                        xsb = pool.tile([P, 1024], F16, tag="xstage")
                        nc.sync.dma_start(
                            out=xsb[:],
                            in_=xparam[kd * P : (kd + 1) * P,
                                       th * 1024 : (th + 1) * 1024],
                        )
                        for dot in range(4):
                            for tc2 in range(2):
                                nc.tensor.matmul(
                                    psums[(dot, tc2)][:],
                                    wsb[:, kd * DH + dot * P : kd * DH + (dot + 1) * P],
                                    xsb[:, tc2 * 512 : (tc2 + 1) * 512],
                                    start=(kd == 0),
                                    stop=(kd == 7),
                                )
                    for dot in range(4):
                        for tc2 in range(2):
                            evac(psums[(dot, tc2)], dot, th * 1024 + tc2 * 512)

            with (
                tc.tile_pool(name="projp", bufs=4) as jp,
                tc.tile_pool(name="projw", bufs=2) as jw,
                tc.tile_pool(name="projpsum", bufs=8, space="PSUM") as jps,
            ):
                wsb = load_w(jw, Wq)

                def evac_q(ps, dot, col):
                    nc.vector.tensor_scalar(
                        quTp0[0:DK, dot * S + col : dot * S + col + 512],
                        ps[0:DK, :], ub_sb[0:DK, dot : dot + 1], None, OP.add)
                    nc.vector.tensor_scalar(
                        quTp1[DK:P, dot * S + col : dot * S + col + 512],
                        ps[DK:P, :], ub_sb[DK:P, dot : dot + 1], None, OP.add)
                    nc.vector.tensor_scalar(
                        qvTp0[0:DK, dot * S + col : dot * S + col + 512],
                        quTp0[0:DK, dot * S + col : dot * S + col + 512],
                        vbmu_sb[0:DK, dot : dot + 1], None, OP.add)
                    nc.vector.tensor_scalar(
                        qvTp1[DK:P, dot * S + col : dot * S + col + 512],
                        quTp1[DK:P, dot * S + col : dot * S + col + 512],
                        vbmu_sb[DK:P, dot : dot + 1], None, OP.add)

                proj_T(jp, jps, wsb, qT, S, evac_q)

                wsb = load_w(jw, Wk)

                def evac_k(ps, dot, col):
                    eng = nc.vector.tensor_copy if dot % 2 == 0 \
                        else nc.scalar.copy
                    eng(khT[:, dot * T + col : dot * T + col + 512], ps[:])

                proj_T(jp, jps, wsb, kT, T, evac_k)

                wsb = load_w(jw, Wr)

                # rh2T[:, dot*3072 + m] = rh[:, (m + 1023) % 2048]; evacuate
                # each 512-col psum chunk straight into its wrapped slots
                def evac_r(ps, dot, col):
                    base = dot * 3072
                    eng = nc.vector.tensor_copy if dot % 2 == 0 \
                        else nc.scalar.copy
                    if col == 0:
                        eng(rh2T[:, base + 1025 : base + 1537], ps[:])
                    elif col == 512:
                        eng(rh2T[:, base + 1537 : base + 2049], ps[:])
                        eng(rh2T[:, base : base + 1], ps[:, 511:512])
                    elif col == 1024:
                        eng(rh2T[:, base + 1 : base + 513], ps[:])
                        eng(rh2T[:, base + 2049 : base + 2561], ps[:])
                    else:
                        eng(rh2T[:, base + 513 : base + 1025], ps[:])
                        eng(rh2T[:, base + 2561 : base + 3072], ps[:, 0:511])

                proj_T(jp, jps, wsb, RT, T, evac_r)

                # vh (untransposed): per key tile tt, psum [128 keys, 512 dh]
                wsb = load_w(jw, Wv)
                for tg in range(2):
                    vps = {tl: jps.tile([P, 512], F32, tag="proj", name="vhps")
                           for tl in range(8)}
                    for kd in range(8):
                        vsb = jp.tile([P, 1024], F16, tag="xstage")
                        nc.sync.dma_start(
                            out=vsb[:],
                            in_=vT[kd * P : (kd + 1) * P,
                                   tg * 1024 : (tg + 1) * 1024],
                        )
                        for tl in range(8):
                            nc.tensor.matmul(
                                vps[tl][:],
                                vsb[:, tl * P : (tl + 1) * P],
                                wsb[:, kd * DH : (kd + 1) * DH],
                                start=(kd == 0),
                                stop=(kd == 7),
                            )
                    for tl in range(8):
                        tt = tg * 8 + tl
                        ps = vps[tl]
                        base = tt * (HC * P)
                        dst = bass.AP(vh128.tensor, vh128.offset + base,
                                      [[vh128.tensor.shape[1], P], [P, HC], [1, DK]])
                        if tl % 2 == 0:
                            nc.vector.tensor_copy(
                                dst, ps[:].rearrange("p (h c) -> p h c", h=HC))
                        else:
                            nc.scalar.copy(
                                dst, ps[:].rearrange("p (h c) -> p h c", h=HC))
                        ones = bass.AP(vh128.tensor, vh128.offset + base + DK,
                                       [[vh128.tensor.shape[1], P], [P, HC]])
                        nc.vector.memset(ones, 1.0)

            # ---------------- attention ----------------
            with (
                tc.tile_pool(name="att_m", bufs=3) as mp,
                tc.tile_pool(name="att_bd", bufs=8) as bdp,
                tc.tile_pool(name="att_att", bufs=2) as atp,
                tc.tile_pool(name="dram", bufs=8, space="DRAM") as dp,
                tc.tile_pool(name="ps_m", bufs=2, space="PSUM") as psm,
                tc.tile_pool(name="ps_sc", bufs=3, space="PSUM") as pssc,
                tc.tile_pool(name="ps_o", bufs=3, space="PSUM") as pso,
                tc.tile_pool(name="smalls", bufs=1) as smp,
            ):
                def prep_pair(hp, a):
                    """M matmuls + HBM shear round trip for both query tiles
                    and both heads of pair a. Returns the sheared bd tiles."""
                    bds = {}
                    for h in range(2):
                        for qi in (2 * a, 2 * a + 1):
                            KQ = kq_of(qi)
                            W = KQ + 127
                            msb = mp.tile([P, WMAX], F16, tag="msb")
                            qvTp = qvTp0 if h == 0 else qvTp1
                            nwc = (W + 511) // 512
                            for wc in range(nwc):
                                nw = min(512, W - wc * 512)
                                mps = psm.tile([P, 512], F32, tag="mps")
                                nc.tensor.matmul(
                                    mps[:, :nw],
                                    qvTp[:, hp * S + qi * P : hp * S + (qi + 1) * P],
                                    rh2T[:, hp * 3072 + qi * P + wc * 512 :
                                         hp * 3072 + qi * P + wc * 512 + nw],
                                    start=True, stop=True,
                                )
                                if wc % 3 != 1:
                                    nc.vector.tensor_copy(
                                        msb[:, wc * 512 : wc * 512 + nw],
                                        mps[:, :nw])
                                else:
                                    nc.scalar.copy(
                                        msb[:, wc * 512 : wc * 512 + nw],
                                        mps[:, :nw])
                            # shear via HBM: write rows stride W, read stride W+1
                            mdr = dp.tile([SLOT], F16, tag="mscr")
                            nc.sync.dma_start(
                                out=bass.AP(mdr.tensor, mdr.offset, [[W, P], [1, W]]),
                                in_=msb[:, :W],
                            )
                            bd = bdp.tile([P, T], F16, tag="bd")
                            nc.scalar.dma_start(
                                out=bd[:, :KQ],
                                in_=bass.AP(mdr.tensor, mdr.offset,
                                            [[W + 1, P], [1, KQ]]),
                            )
                            bds[(qi, h)] = bd
                    return bds

                def recip_quad(ovs):
                    # both heads' sums rows at partitions 0/64 -> one wide
                    # approx reciprocal (rows 1-63 garbage, unused)
                    sums2 = smp.tile([P, 512], F32, tag="sums2")
                    rc16 = smp.tile([P, 512], F16, tag="rc16")
                    for h in range(2):
                        nc.scalar.copy(sums2[64 * h : 64 * h + 1, :],
                                       ovs[h][64:65, :])
                    with nc.allow_low_precision("softmax denominators"):
                        nc.vector.reciprocal(rc16[0:65, :], sums2[0:65, :])
                    return rc16

                def finish_quad(hp, g, ovs, rc16):
                    for h in range(2):
                        rb = psm.tile([P, 512], F32, tag="mps", name="rb")
                        nc.tensor.matmul(
                            rb[:DK, :],
                            ones16[64 * h : 64 * h + 1, :],
                            rc16[64 * h : 64 * h + 1, :],
                            start=True, stop=True)
                        rbsb = smp.tile([P, 512], F16, tag="rbsb")
                        nc.scalar.copy(rbsb[:DK, :], rb[:DK, :])
                        nc.vector.tensor_tensor(
                            concatT[h * DK : (h + 1) * DK,
                                    hp * S + g * 512 : hp * S + (g + 1) * 512],
                            ovs[h][0:DK, :], rbsb[0:DK, :], OP.mult)

                def score_quad(hp, g, bds):
                    """Score + AV for query tiles 4g..4g+3, both heads.
                    att^T layout: per key tile jt a 512-wide block with one
                    128-col slot per query tile of the quad."""
                    njts = [kq_of(4 * g + qs) // P for qs in range(4)]
                    njt3 = njts[3]
                    atts = {}
                    for h in range(2):
                        pr = slice(h * DK, (h + 1) * DK)
                        att = atp.tile([P, 16 * 512], F16, tag="attTQ",
                                       name="attTQ")
                        for jt in range(njt3):
                            scT = pssc.tile([P, 512], F32, tag="scT")
                            # content term for all 4 query tiles at once
                            # (start resets the whole 512-wide chunk)
                            quTp = quTp0 if h == 0 else quTp1
                            nc.tensor.matmul(
                                scT[:],
                                khT[:, hp * T + jt * P : hp * T + (jt + 1) * P],
                                quTp[:, hp * S + 4 * g * P :
                                     hp * S + (4 * g + 4) * P],
                                start=True, stop=False,
                            )
                            # position term: bd^T per valid quarter via a
                            # plain matmul against the identity
                            valid = [qs for qs in range(4) if jt < njts[qs]]
                            for vi, qs in enumerate(valid):
                                nc.tensor.matmul(
                                    scT[:, qs * P : (qs + 1) * P],
                                    bds[(4 * g + qs, h)][:, jt * P : (jt + 1) * P],
                                    ident_sb[:],
                                    start=False, stop=(vi == len(valid) - 1),
                                )
                            nc.scalar.activation(
                                att[:, jt * 512 : (jt + 1) * 512], scT[:],
                                AF.Exp, bias=expb_sb[:], scale=0.125)
                        # masks: causal boundary triangle per query tile, and
                        # zero the slots past each tile's key range
                        for qs in range(4):
                            njt = njts[qs]
                            nc.vector.copy_predicated(
                                att[:, (njt - 1) * 512 + qs * P :
                                    (njt - 1) * 512 + qs * P + P],
                                masklo_sb[:], zeros16[:])
                            for jt in range(njt, njt3):
                                nc.vector.memset(
                                    att[:, jt * 512 + qs * P :
                                        jt * 512 + qs * P + P], 0.0)

                        atts[h] = att
                    return atts

                def av_quad(hp, g, atts):
                    njt3 = kq_of(4 * g + 3) // P
                    ovs = {}
                    for h in range(2):
                        att = atts[h]
                        # AV, unnormalized; vh ones-row gives sums in row 64
                        ov = pso.tile([P, 512], F32, tag="ops")
                        for jt in range(njt3):
                            nc.tensor.matmul(
                                ov[:, :],
                                vh128[:, jt * (HC * P) + (hp * 2 + h) * P :
                                      jt * (HC * P) + (hp * 2 + h) * P + P],
                                att[:, jt * 512 : (jt + 1) * 512],
                                start=(jt == 0), stop=(jt == njt3 - 1),
                            )
                        ovs[h] = ov
                    return ovs

                seq = [(hp, a) for hp in range(4) for a in range(4)]
                quads = [(hp, g) for hp in range(4) for g in range(2)]
                prepped = {}

                def do_prep(i):
                    hp, a = seq[i]
                    for (qi, h), bd in prep_pair(hp, a).items():
                        prepped[(qi, h, hp)] = bd

                do_prep(0)
                do_prep(1)
                pending_fin = None
                for i, (hp, g) in enumerate(quads):
                    if 2 * i + 2 < len(seq):
                        do_prep(2 * i + 2)
                    bds = {(qi, h): prepped.pop((qi, h, hp))
                           for qs in range(4) for h in range(2)
                           for qi in [4 * g + qs]}
                    atts = score_quad(hp, g, bds)
                    if pending_fin is not None:
                        finish_quad(*pending_fin)
                    ovs = av_quad(hp, g, atts)
                    rc16 = recip_quad(ovs)
                    pending_fin = (hp, g, ovs, rc16)
                    if 2 * i + 3 < len(seq):
                        do_prep(2 * i + 3)
                finish_quad(*pending_fin)

            # ---------------- output projection ----------------
            with (
                tc.tile_pool(name="outp", bufs=2) as op_,
                tc.tile_pool(name="outw", bufs=1) as opw,
                tc.tile_pool(name="outpsum", bufs=4, space="PSUM") as ops_,
            ):
                # WoS layout [128, dt*1024 + o] <- Wo16[(dt p), o]
                WoS = opw.tile([P, 4 * D], F16)
                for dt_ in range(4):
                    nc.sync.dma_start(
                        out=WoS[:, dt_ * D : (dt_ + 1) * D],
                        in_=Wo16[dt_ * P : (dt_ + 1) * P, :],
                    )
                for it in range(8):
                    for oc in range(2):
                        ps = ops_.tile([P, 512], F32, tag="out")
                        for dt in range(4):
                            nc.tensor.matmul(
                                ps[:],
                                concatT[:, dt * S + it * P : dt * S + (it + 1) * P],
                                WoS[:, dt * D + oc * 512 : dt * D + (oc + 1) * 512],
                                start=(dt == 0), stop=(dt == 3),
                            )
                        osb = op_.tile([P, 512], F32, tag="osb")
                        nc.vector.tensor_copy(osb[:], ps[:])
                        nc.sync.dma_start(
                            out=outp[it * P : (it + 1) * P, oc * 512 : (oc + 1) * 512],
                            in_=osb[:])

    if split_waits:
        _split_multi_waits(nc)
    return nc


def prep_core_inputs(core, q, k, v, u, v_bias, Wq, Wk, Wv, Wr, Wo, R):
    b, hh = core // 2, core % 2
    sl = slice(hh * DH, (hh + 1) * DH)
    c = np.ascontiguousarray
    f16 = np.float16
    return {
        "qT": c(q[b].T).astype(f16),
        "kT": c(k[b].T).astype(f16),
        "vT": c(v[b].T).astype(f16),
        "RT": c(R.T).astype(f16),
        "Wq": c(Wq[sl, :].T).astype(f16),
        "Wk": c(Wk[sl, :].T).astype(f16),
        "Wv": c(Wv[sl, :].T).astype(f16),
        "Wr": c(Wr[sl, :].T).astype(f16),
        "Wo16": c(Wo[:, sl].T).astype(f16),
        "ub": c(u[0, hh * HC : (hh + 1) * HC, 0, :].reshape(4, P).T),
        "vb": c(v_bias[0, hh * HC : (hh + 1) * HC, 0, :].reshape(4, P).T),
        "ident": np.eye(P, dtype=f16),
        "masklo": np.tril(np.ones((P, P), np.uint8), k=-1),
    }


def combine_outputs(results):
    # results: list of 8 dicts with "out" [S, D]; partial sums per batch pair
    out = np.empty((4, S, D), np.float32)
    for b in range(4):
        out[b] = results[2 * b]["out"] + results[2 * b + 1]["out"]
    return out


_CACHED_NC = None
last_result = None  # BassKernelResults of the most recent run (for test harness)


def kernel(q, k, v, mask, u, v_bias, Wq, Wk, Wv, Wr, Wo, R):
    global _CACHED_NC, last_result
    from concourse.bass_utils import run_bass_kernel_spmd

    q, k, v = np.asarray(q), np.asarray(k), np.asarray(v)
    u, v_bias = np.asarray(u), np.asarray(v_bias)
    Wq, Wk, Wv, Wr, Wo, R = map(np.asarray, (Wq, Wk, Wv, Wr, Wo, R))

    # The kernel exploits the known TXL mask structure (j <= i + MEM).
    # Verify the passed mask matches; structural masking is baked in.
    m = np.asarray(mask)
    exp_mask = (np.arange(T)[None, :] <= np.arange(S)[:, None] + 1024)
    assert m.shape == (4, S, T) and bool((m == exp_mask[None]).all()), \
        "kernel compiled for the TXL causal mask (j <= i + MEM)"

    if _CACHED_NC is None:
        _CACHED_NC = build_nc()

    in_maps = [prep_core_inputs(c, q, k, v, u, v_bias, Wq, Wk, Wv, Wr, Wo, R)
               for c in range(8)]
    last_result = run_bass_kernel_spmd(_CACHED_NC, in_maps, list(range(8)))
    return combine_outputs(last_result.results)


# revision 37
# speedup vs baseline: 1.1081x; 1.1081x over previous
"""Transformer-XL relative multi-head attention, 8-way sharded on Trainium2.

Self-contained harness entry: kernel(**inputs) -> np.ndarray [4, 1024, 1024].

Sharding: core c handles batch b = c//2 and head-half hh = c%2 (8 of 16
heads). Each core computes a partial output (its heads' contribution
through Wo); the host unshard sums the two partials per batch (row-parallel
tensor parallelism for the output projection).

Attention flow (per head, per 128-row query tile):
  - position term M = (qh+v_bias) @ rh2^T  -> HBM shear (write stride W,
    read stride W+1) gives the circulant-shifted bd in [q, keys] layout.
  - per 128-key tile: PE-transpose of the bd tile into PSUM (start=True),
    then the content term k^T q accumulated on top (start=False), so the
    PSUM holds score^T = (ac+bd)^T without any DVE adds or DMA transposes.
  - exp on the Scalar engine straight out of PSUM into att^T in SBUF.
  - AV contracts key tiles; vh carries an extra ones-row so row 64 of the
    AV output is the softmax denominator; a 1-partition PE matmul
    broadcasts the reciprocal row and the AV evacuation multiplies by it.
"""

import os
import sys

sys.path.insert(0, "/opt/trn_rl_repo")

import numpy as np


import concourse.bass as bass
import concourse.mybir as mybir
from concourse.tile import TileContext, ScopedClock

F32 = mybir.dt.float32
F16 = mybir.dt.float16
AF = mybir.ActivationFunctionType
OP = mybir.AluOpType

S, T, D, HC, DK, P = 1024, 2048, 1024, 8, 64, 128
DH = HC * DK  # 512, head-slice width per core
NQT = S // P  # 8 query tiles
WMAX = 2048 + 128  # padded shear slot width (>= max W = 2175)
SLOT = P * WMAX + P  # dram scratch slot elements (pad for strided read)
EXP_BIAS = -7.0


def _patched_drain_and_barrier(self, tick_clock, wait_clock):
    # The walrus build in this container caps sync-waits per instruction;
    # Tile's stock tail drain carries one wait per live proc. Emit one SP nop
    # per wait instead, then the drain.
    dummy = mybir.InstNoOp(name="drain-wait-probe", ins=[], outs=[])
    dummy.engine = mybir.EngineType.SP
    wait_clock.add_sem_waits(dummy, ScopedClock({None: tick_clock.global_clock}))
    waits = []
    if dummy.sync_info is not None and dummy.sync_info.on_wait:
        waits = [(w.ant_name, w.wait_value) for w in dummy.sync_info.on_wait]
    assert self.sems is not None
    name2sem = {h.name: h for h in self.sems.allocated().values()}
    for name, val in waits:
        self.nc.sync.nop().wait_op(name2sem[name], val, "sem-ge")
    self.nc.sync.drain()
    self.nc.all_engine_barrier()
    popped = self.nc._tile_sem_poison_stack.pop()
    assert popped is self._sem_poison
    self.nc.clear_and_free_semaphores(list(self.sems.allocated().values()))
    self.nc.all_engine_barrier()


TileContext._drain_and_barrier = _patched_drain_and_barrier



def _split_multi_waits(nc, max_waits=1):
    """Walrus in this container rejects instructions carrying more than a
    couple of sync waits. Hoist extras onto same-engine NoOps just before
    the instruction (sequential on the engine, so semantics unchanged)."""
    for f in nc.m.functions:
        for bb in f.blocks:
            out = []
            changed = False
            for inst in bb.instructions:
                si = inst.sync_info
                if si is not None and si.on_wait and len(si.on_wait) > max_waits:
                    waits = list(si.on_wait)
                    for j, w in enumerate(waits[:-max_waits]):
                        nop = mybir.InstNoOp(
                            name=f"{inst.name}-wsplit{j}", ins=[], outs=[])
                        nop.engine = inst.engine
                        nop.sync_info = mybir.SyncInfo(on_wait=[w], on_update=[])
                        out.append(nop)
                    inst.sync_info = mybir.SyncInfo(
                        on_wait=waits[-max_waits:],
                        on_update=list(si.on_update))
                    changed = True
                out.append(inst)
            if changed:
                bb.instructions = out


def kq_of(qi):  # valid key count for query tile qi (keys j <= i + 1024)
    return (qi + 9) * P


def build_nc(split_waits=True):
    nc = bass.Bass(target_bir_lowering=True)

    qT = nc.declare_dram_parameter("qT", [D, S], F16, isOutput=False)
    kT = nc.declare_dram_parameter("kT", [D, T], F16, isOutput=False)
    vT = nc.declare_dram_parameter("vT", [D, T], F16, isOutput=False)
    RT = nc.declare_dram_parameter("RT", [D, T], F16, isOutput=False)
    Wq = nc.declare_dram_parameter("Wq", [D, DH], F16, isOutput=False)
    Wk = nc.declare_dram_parameter("Wk", [D, DH], F16, isOutput=False)
    Wv = nc.declare_dram_parameter("Wv", [D, DH], F16, isOutput=False)
    Wr = nc.declare_dram_parameter("Wr", [D, DH], F16, isOutput=False)
    Wo16 = nc.declare_dram_parameter("Wo16", [DH, D], F16, isOutput=False)
    ub = nc.declare_dram_parameter("ub", [P, 4], F32, isOutput=False)
    vb = nc.declare_dram_parameter("vb", [P, 4], F32, isOutput=False)
    ident = nc.declare_dram_parameter("ident", [P, P], F16, isOutput=False)
    masklo = nc.declare_dram_parameter(
        "masklo", [P, P], mybir.dt.uint8, isOutput=False)
    outp = nc.declare_dram_parameter("out", [S, D], F32, isOutput=True)

    with TileContext(nc) as tc:
        with (
            tc.tile_pool(name="persist", bufs=1) as pp,
            tc.tile_pool(name="consts", bufs=1) as cp,
        ):
            # persistent fp16 tensors (partition = dk within head-pair tile).
            # Matmuls only hit full rate with K=128 and M=128, so per-head
            # stationaries are zero-padded to all 128 partitions: the padded
            # rows multiply the other head's moving rows by zero.
            quTp0 = pp.tile([P, 4 * S], F16)    # (qh+u).T head-even, rows 64+ zero
            quTp1 = pp.tile([P, 4 * S], F16)    # head-odd, rows 0-63 zero
            qvTp0 = pp.tile([P, 4 * S], F16)    # (qh+v).T head-even, rows 64+ zero
            qvTp1 = pp.tile([P, 4 * S], F16)    # head-odd, rows 0-63 zero
            khT = pp.tile([P, 4 * T], F16)
            rh2T = pp.tile([P, 4 * 3072], F16)
            vh128 = pp.tile([P, 16 * (HC * P)], F16)  # tile block: 8 heads x 128
            concatT = pp.tile([P, 4 * S], F16)

            ub_sb = cp.tile([P, 4], F32)
            vb_sb = cp.tile([P, 4], F32)
            vbmu_sb = cp.tile([P, 4], F32)
            ident_sb = cp.tile([P, P], F16)
            masklo_sb = cp.tile([P, P], mybir.dt.uint8)
            zeros16 = cp.tile([P, P], F16)
            ones16 = cp.tile([P, DK], F16)
            expb_sb = cp.tile([P, 1], F32)
            nc.vector.memset(expb_sb[:], EXP_BIAS)
            nc.vector.memset(zeros16[:], 0.0)
            nc.vector.memset(ones16[:], 1.0)

            nc.sync.dma_start(out=ub_sb[:], in_=ub[:])
            nc.sync.dma_start(out=vb_sb[:], in_=vb[:])
            nc.vector.tensor_tensor(vbmu_sb[:], vb_sb[:], ub_sb[:], OP.subtract)
            nc.sync.dma_start(out=ident_sb[:], in_=ident[:])
            nc.sync.dma_start(out=masklo_sb[:], in_=masklo[:])
            nc.vector.memset(quTp0[DK:P, :], 0.0)
            nc.vector.memset(quTp1[0:DK, :], 0.0)
            nc.vector.memset(qvTp0[DK:P, :], 0.0)
            nc.vector.memset(qvTp1[0:DK, :], 0.0)
            nc.vector.memset(vh128[:], 0.0)

            # ---------------- projections ----------------
            def load_w(pool, wparam):
                wsb = pool.tile([P, 8 * DH], F16, tag="wsb")
                nc.sync.dma_start(
                    out=bass.AP(wsb.tensor, wsb.offset,
                                [[wsb.tensor.shape[1], P], [DH, 8], [1, DH]]),
                    in_=bass.AP(wparam, 0, [[DH, P], [P * DH, 8], [1, DH]]),
                )
                return wsb

            # qhT-style projection: out[512, ncols] = W_s @ xT, evacuated by fn
            def proj_T(pool, psum, wsb, xparam, ncols, evac):
                nth = ncols // 1024
                for th in range(nth):
                    psums = {k: psum.tile([P, 512], F32, tag="proj", name="projps")
                             for k in [(d, t2) for d in range(4) for t2 in range(2)]}
                    for kd in range(8):
                        xsb = pool.tile([P, 1024], F16, tag="xstage")
                        nc.sync.dma_start(
                            out=xsb[:],
                            in_=xparam[kd * P : (kd + 1) * P,
                                       th * 1024 : (th + 1) * 1024],
                        )
                        for dot in range(4):
                            for tc2 in range(2):
                                nc.tensor.matmul(
                                    psums[(dot, tc2)][:],
                                    wsb[:, kd * DH + dot * P : kd * DH + (dot + 1) * P],
                                    xsb[:, tc2 * 512 : (tc2 + 1) * 512],
                                    start=(kd == 0),
                                    stop=(kd == 7),
                                )
                    for dot in range(4):
                        for tc2 in range(2):
                            evac(psums[(dot, tc2)], dot, th * 1024 + tc2 * 512)

            with (
                tc.tile_pool(name="projp", bufs=3) as jp,
                tc.tile_pool(name="projw", bufs=2) as jw,
                tc.tile_pool(name="projpsum", bufs=8, space="PSUM") as jps,
            ):
                wsb = load_w(jw, Wq)

                def evac_q(ps, dot, col):
                    nc.vector.tensor_scalar(
                        quTp0[0:DK, dot * S + col : dot * S + col + 512],
                        ps[0:DK, :], ub_sb[0:DK, dot : dot + 1], None, OP.add)
                    nc.vector.tensor_scalar(
                        quTp1[DK:P, dot * S + col : dot * S + col + 512],
                        ps[DK:P, :], ub_sb[DK:P, dot : dot + 1], None, OP.add)
                    nc.vector.tensor_scalar(
                        qvTp0[0:DK, dot * S + col : dot * S + col + 512],
                        quTp0[0:DK, dot * S + col : dot * S + col + 512],
                        vbmu_sb[0:DK, dot : dot + 1], None, OP.add)
                    nc.vector.tensor_scalar(
                        qvTp1[DK:P, dot * S + col : dot * S + col + 512],
                        quTp1[DK:P, dot * S + col : dot * S + col + 512],
                        vbmu_sb[DK:P, dot : dot + 1], None, OP.add)

                proj_T(jp, jps, wsb, qT, S, evac_q)

                wsb = load_w(jw, Wk)

                def evac_k(ps, dot, col):
                    eng = nc.vector.tensor_copy if dot % 2 == 0 \
                        else nc.scalar.copy
                    eng(khT[:, dot * T + col : dot * T + col + 512], ps[:])

                proj_T(jp, jps, wsb, kT, T, evac_k)

                wsb = load_w(jw, Wr)

                # rh2T[:, dot*3072 + m] = rh[:, (m + 1023) % 2048]; evacuate
                # each 512-col psum chunk straight into its wrapped slots
                def evac_r(ps, dot, col):
                    base = dot * 3072
                    eng = nc.vector.tensor_copy if dot % 2 == 0 \
                        else nc.scalar.copy
                    if col == 0:
                        eng(rh2T[:, base + 1025 : base + 1537], ps[:])
                    elif col == 512:
                        eng(rh2T[:, base + 1537 : base + 2049], ps[:])
                        eng(rh2T[:, base : base + 1], ps[:, 511:512])
                    elif col == 1024:
                        eng(rh2T[:, base + 1 : base + 513], ps[:])
                        eng(rh2T[:, base + 2049 : base + 2561], ps[:])
                    else:
                        eng(rh2T[:, base + 513 : base + 1025], ps[:])
                        eng(rh2T[:, base + 2561 : base + 3072], ps[:, 0:511])

                proj_T(jp, jps, wsb, RT, T, evac_r)

                # vh (untransposed): per key tile tt, psum [128 keys, 512 dh]
                wsb = load_w(jw, Wv)
                for tg in range(2):
                    vps = {tl: jps.tile([P, 512], F32, tag="proj", name="vhps")
                           for tl in range(8)}
                    for kd in range(8):
                        vsb = jp.tile([P, 1024], F16, tag="xstage")
                        nc.sync.dma_start(
                            out=vsb[:],
                            in_=vT[kd * P : (kd + 1) * P,
                                   tg * 1024 : (tg + 1) * 1024],
                        )
                        for tl in range(8):
                            nc.tensor.matmul(
                                vps[tl][:],
                                vsb[:, tl * P : (tl + 1) * P],
                                wsb[:, kd * DH : (kd + 1) * DH],
                                start=(kd == 0),
                                stop=(kd == 7),
                            )
                    for tl in range(8):
                        tt = tg * 8 + tl
                        ps = vps[tl]
                        base = tt * (HC * P)
                        dst = bass.AP(vh128.tensor, vh128.offset + base,
                                      [[vh128.tensor.shape[1], P], [P, HC], [1, DK]])
                        if tl % 2 == 0:
                            nc.vector.tensor_copy(
                                dst, ps[:].rearrange("p (h c) -> p h c", h=HC))
                        else:
                            nc.scalar.copy(
                                dst, ps[:].rearrange("p (h c) -> p h c", h=HC))
                        ones = bass.AP(vh128.tensor, vh128.offset + base + DK,
                                       [[vh128.tensor.shape[1], P], [P, HC]])
                        nc.vector.memset(ones, 1.0)

            # ---------------- attention ----------------
            with (
                tc.tile_pool(name="att_m", bufs=3) as mp,
                tc.tile_pool(name="att_bd", bufs=8) as bdp,
                tc.tile_pool(name="att_att", bufs=2) as atp,
                tc.tile_pool(name="dram", bufs=8, space="DRAM") as dp,
                tc.tile_pool(name="ps_m", bufs=2, space="PSUM") as psm,
                tc.tile_pool(name="ps_sc", bufs=2, space="PSUM") as pssc,
                tc.tile_pool(name="ps_o", bufs=3, space="PSUM") as pso,
                tc.tile_pool(name="ps_rb", bufs=1, space="PSUM") as psrb,
                tc.tile_pool(name="smalls", bufs=1) as smp,
            ):
                def prep_pair(hp, a):
                    """M matmuls + HBM shear round trip for both query tiles
                    and both heads of pair a. Returns the sheared bd tiles."""
                    bds = {}
                    for h in range(2):
                        for qi in (2 * a, 2 * a + 1):
                            KQ = kq_of(qi)
                            W = KQ + 127
                            msb = mp.tile([P, WMAX], F16, tag="msb")
                            qvTp = qvTp0 if h == 0 else qvTp1
                            nwc = (W + 511) // 512
                            for wc in range(nwc):
                                nw = min(512, W - wc * 512)
                                mps = psm.tile([P, 512], F32, tag="mps")
                                nc.tensor.matmul(
                                    mps[:, :nw],
                                    qvTp[:, hp * S + qi * P : hp * S + (qi + 1) * P],
                                    rh2T[:, hp * 3072 + qi * P + wc * 512 :
                                         hp * 3072 + qi * P + wc * 512 + nw],
                                    start=True, stop=True,
                                )
                                if wc % 3 != 1:
                                    nc.vector.tensor_copy(
                                        msb[:, wc * 512 : wc * 512 + nw],
                                        mps[:, :nw])
                                else:
                                    nc.scalar.copy(
                                        msb[:, wc * 512 : wc * 512 + nw],
                                        mps[:, :nw])
                            # shear via HBM: write rows stride W, read stride W+1
                            mdr = dp.tile([SLOT], F16, tag="mscr")
                            nc.sync.dma_start(
                                out=bass.AP(mdr.tensor, mdr.offset, [[W, P], [1, W]]),
                                in_=msb[:, :W],
                            )
                            bd = bdp.tile([P, T], F16, tag="bd")
                            nc.scalar.dma_start(
                                out=bd[:, :KQ],
                                in_=bass.AP(mdr.tensor, mdr.offset,
                                            [[W + 1, P], [1, KQ]]),
                            )
                            bds[(qi, h)] = bd
                    return bds

                def recip_quad(ovs):
                    # both heads' sums rows at partitions 0/64 -> one wide
                    # approx reciprocal (rows 1-63 garbage, unused)
                    sums2 = smp.tile([P, 512], F32, tag="sums2")
                    rc16 = smp.tile([P, 512], F16, tag="rc16")
                    for h in range(2):
                        nc.scalar.copy(sums2[64 * h : 64 * h + 1, :],
                                       ovs[h][64:65, :])
                    with nc.allow_low_precision("softmax denominators"):
                        nc.vector.reciprocal(rc16[0:65, :], sums2[0:65, :])
                    return rc16

                def finish_quad(hp, g, ovs, rc16):
                    for h in range(2):
                        rb = psrb.tile([P, 512], F32, tag="rb")
                        nc.tensor.matmul(
                            rb[:DK, :],
                            ones16[64 * h : 64 * h + 1, :],
                            rc16[64 * h : 64 * h + 1, :],
                            start=True, stop=True)
                        rbsb = smp.tile([P, 512], F16, tag="rbsb")
                        nc.scalar.copy(rbsb[:DK, :], rb[:DK, :])
                        nc.vector.tensor_tensor(
                            concatT[h * DK : (h + 1) * DK,
                                    hp * S + g * 512 : hp * S + (g + 1) * 512],
                            ovs[h][0:DK, :], rbsb[0:DK, :], OP.mult)

                def score_quad(hp, g, bds):
                    """Score + AV for query tiles 4g..4g+3, both heads.
                    att^T layout: per key tile jt a 512-wide block with one
                    128-col slot per query tile of the quad."""
                    njts = [kq_of(4 * g + qs) // P for qs in range(4)]
                    njt3 = njts[3]
                    atts = {}
                    for h in range(2):
                        pr = slice(h * DK, (h + 1) * DK)
                        att = atp.tile([P, 16 * 512], F16, tag="attTQ",
                                       name="attTQ")
                        for jt in range(njt3):
                            scT = pssc.tile([P, 512], F32, tag="scT")
                            # content term for all 4 query tiles at once
                            # (start resets the whole 512-wide chunk)
                            quTp = quTp0 if h == 0 else quTp1
                            nc.tensor.matmul(
                                scT[:],
                                khT[:, hp * T + jt * P : hp * T + (jt + 1) * P],
                                quTp[:, hp * S + 4 * g * P :
                                     hp * S + (4 * g + 4) * P],
                                start=True, stop=False,
                            )
                            # position term: bd^T per valid quarter via a
                            # plain matmul against the identity
                            valid = [qs for qs in range(4) if jt < njts[qs]]
                            for vi, qs in enumerate(valid):
                                nc.tensor.matmul(
                                    scT[:, qs * P : (qs + 1) * P],
                                    bds[(4 * g + qs, h)][:, jt * P : (jt + 1) * P],
                                    ident_sb[:],
                                    start=False, stop=(vi == len(valid) - 1),
                                )
                            nc.scalar.activation(
                                att[:, jt * 512 : (jt + 1) * 512], scT[:],
                                AF.Exp, bias=expb_sb[:], scale=0.125)
                        # masks: causal boundary triangle per query tile, and
                        # zero the slots past each tile's key range
                        for qs in range(4):
                            njt = njts[qs]
                            nc.vector.copy_predicated(
                                att[:, (njt - 1) * 512 + qs * P :
                                    (njt - 1) * 512 + qs * P + P],
                                masklo_sb[:], zeros16[:])
                            for jt in range(njt, njt3):
                                nc.vector.memset(
                                    att[:, jt * 512 + qs * P :
                                        jt * 512 + qs * P + P], 0.0)

                        atts[h] = att
                    return atts

                def av_quad(hp, g, atts):
                    njt3 = kq_of(4 * g + 3) // P
                    ovs = {}
                    for h in range(2):
                        att = atts[h]
                        # AV, unnormalized; vh ones-row gives sums in row 64
                        ov = pso.tile([P, 512], F32, tag="ops")
                        for jt in range(njt3):
                            nc.tensor.matmul(
                                ov[:, :],
                                vh128[:, jt * (HC * P) + (hp * 2 + h) * P :
                                      jt * (HC * P) + (hp * 2 + h) * P + P],
                                att[:, jt * 512 : (jt + 1) * 512],
                                start=(jt == 0), stop=(jt == njt3 - 1),
                            )
                        ovs[h] = ov
                    return ovs

                seq = [(hp, a) for hp in range(4) for a in range(4)]
                quads = [(hp, g) for hp in range(4) for g in range(2)]
                prepped = {}

                def do_prep(i):
                    hp, a = seq[i]
                    for (qi, h), bd in prep_pair(hp, a).items():
                        prepped[(qi, h, hp)] = bd

                do_prep(0)
                do_prep(1)
                pending_fin = None
                for i, (hp, g) in enumerate(quads):
                    if 2 * i + 2 < len(seq):
                        do_prep(2 * i + 2)
                    bds = {(qi, h): prepped.pop((qi, h, hp))
                           for qs in range(4) for h in range(2)
                           for qi in [4 * g + qs]}
                    atts = score_quad(hp, g, bds)
                    if pending_fin is not None:
                        finish_quad(*pending_fin)
                    ovs = av_quad(hp, g, atts)
                    rc16 = recip_quad(ovs)
                    pending_fin = (hp, g, ovs, rc16)
                    if 2 * i + 3 < len(seq):
                        do_prep(2 * i + 3)
                finish_quad(*pending_fin)

            # ---------------- output projection ----------------
            with (
                tc.tile_pool(name="outp", bufs=2) as op_,
                tc.tile_pool(name="outw", bufs=1) as opw,
                tc.tile_pool(name="outpsum", bufs=4, space="PSUM") as ops_,
            ):
                # WoS layout [128, dt*1024 + o] <- Wo16[(dt p), o]
                WoS = opw.tile([P, 4 * D], F16)
                for dt_ in range(4):
                    nc.sync.dma_start(
                        out=WoS[:, dt_ * D : (dt_ + 1) * D],
                        in_=Wo16[dt_ * P : (dt_ + 1) * P, :],
                    )
                for it in range(8):
                    for oc in range(2):
                        ps = ops_.tile([P, 512], F32, tag="out")
                        for dt in range(4):
                            nc.tensor.matmul(
                                ps[:],
                                concatT[:, dt * S + it * P : dt * S + (it + 1) * P],
                                WoS[:, dt * D + oc * 512 : dt * D + (oc + 1) * 512],
                                start=(dt == 0), stop=(dt == 3),
                            )
                        osb = op_.tile([P, 512], F32, tag="osb")
                        nc.vector.tensor_copy(osb[:], ps[:])
                        nc.sync.dma_start(
                            out=outp[it * P : (it + 1) * P, oc * 512 : (oc + 1) * 512],
                            in_=osb[:])

    if split_waits:
        _split_multi_waits(nc)
    return nc


def prep_core_inputs(core, q, k, v, u, v_bias, Wq, Wk, Wv, Wr, Wo, R):
    b, hh = core // 2, core % 2
    sl = slice(hh * DH, (hh + 1) * DH)
    c = np.ascontiguousarray
    f16 = np.float16
    return {
        "qT": c(q[b].T).astype(f16),
        "kT": c(k[b].T).astype(f16),
        "vT": c(v[b].T).astype(f16),
        "RT": c(R.T).astype(f16),
        "Wq": c(Wq[sl, :].T).astype(f16),
        "Wk": c(Wk[sl, :].T).astype(f16),
        "Wv": c(Wv[sl, :].T).astype(f16),
        "Wr": c(Wr[sl, :].T).astype(f16),
        "Wo16": c(Wo[:, sl].T).astype(f16),
        "ub": c(u[0, hh * HC : (hh + 1) * HC, 0, :].reshape(4, P).T),
        "vb": c(v_bias[0, hh * HC : (hh + 1) * HC, 0, :].reshape(4, P).T),
        "ident": np.eye(P, dtype=f16),
        "masklo": np.tril(np.ones((P, P), np.uint8), k=-1),
    }


def combine_outputs(results):
    # results: list of 8 dicts with "out" [S, D]; partial sums per batch pair
    out = np.empty((4, S, D), np.float32)
    for b in range(4):
        out[b] = results[2 * b]["out"] + results[2 * b + 1]["out"]
    return out


_CACHED_NC = None
last_result = None  # BassKernelResults of the most recent run (for test harness)


def kernel(q, k, v, mask, u, v_bias, Wq, Wk, Wv, Wr, Wo, R):
    global _CACHED_NC, last_result
    from concourse.bass_utils import run_bass_kernel_spmd

    q, k, v = np.asarray(q), np.asarray(k), np.asarray(v)
    u, v_bias = np.asarray(u), np.asarray(v_bias)
    Wq, Wk, Wv, Wr, Wo, R = map(np.asarray, (Wq, Wk, Wv, Wr, Wo, R))

    # The kernel exploits the known TXL mask structure (j <= i + MEM).
    # Verify the passed mask matches; structural masking is baked in.
    m = np.asarray(mask)
    exp_mask = (np.arange(T)[None, :] <= np.arange(S)[:, None] + 1024)
    assert m.shape == (4, S, T) and bool((m == exp_mask[None]).all()), \
        "kernel compiled for the TXL causal mask (j <= i + MEM)"

    if _CACHED_NC is None:
        _CACHED_NC = build_nc()

    in_maps = [prep_core_inputs(c, q, k, v, u, v_bias, Wq, Wk, Wv, Wr, Wo, R)
               for c in range(8)]
    last_result = run_bass_kernel_spmd(_CACHED_NC, in_maps, list(range(8)))
    return combine_outputs(last_result.results)
